# revision 37
# baseline (speedup 1.0000x reference)
"""Multi-head attention (B=2, H=8, S=2048, hd=16) on 8 Trainium2 NeuronCores.

Sharding: 16 (batch, head) groups -> 2 heads per core (cores 0-3: batch 0,
cores 4-7: batch 1).  Each core gets transposed embeddings, a key-compacted
copy (keys with source-mask 0 dropped; padded to NK with -1000 mask columns),
and the 32 projection-weight columns for its two heads.

Score matmuls run in float32r (1 cycle/row on the PE vs 4 for fp32) with
fp32-level accuracy recovered via split-precision row packing: K and Q are
each split into bf16-high + fp32-residual parts (Kh+Kl, Qh+Ql) and the four
cross products are packed into one 128-row contraction
  rows  0:16  Kh x Qh        rows 32:48  Kl x Qh(dup)
  rows 64:80  Kh(dup) x Ql   (Kl x Ql dropped: ~4e-4 score error)
  row 16: mask x ones        row 17: ones x (-rowmax)      (gaps zeroed)
Extra contraction rows are free (matmul cost is N output columns only), and
bf16-grid values pass through the PE's f32r truncation unchanged, so the sum
reconstructs the exact fp32 product.

The softmax shift per query row is a host-computed Cauchy-Schwarz upper
bound m = 0.25*|q_row|*max_k|k| - 30: it can never overflow exp (bound >=
true max, 30 margin under fp32's e^88), and underflow of the whole row
requires max-cos < 0.4 over ~1k random keys (P ~ e^-58).  No on-device
row-max pass at all; inf/NaN/zero rows would be repaired on host (safety
net, expected zero).

ctx = P^T @ [V | 1] accumulates in PSUM with f32r operands; the ones column
gives the softmax denominator l; 1/l is computed on a [128,16] reshape (not
the serial [1,2048] row) and applied via DRAM-broadcast + DVE multiply.
Output per core is [32, 2048] (dim-major); the host scatters back into the
interleaved head layout.
"""

import numpy as np

S = 2048
E = 128
HD = 16
NQB = S // 128       # 16 query blocks
NEG = -1000.0
NA = 512             # keys sampled for the row-max pass

_PROGS = {}


def _build_program(NKB):
    import concourse.mybir as mybir
    from concourse import bacc
    from concourse.tile import TileContext

    NK = 128 * NKB

    fp32 = mybir.dt.float32
    f32r = mybir.dt.float32r
    bf16 = mybir.dt.bfloat16
    AF = mybir.ActivationFunctionType
    ALU = mybir.AluOpType
    AX = mybir.AxisListType

    nc = bacc.Bacc()

    xT = nc.declare_dram_parameter("xT", [E, S], fp32, isOutput=False)
    xkT = nc.declare_dram_parameter("xkT", [E, NK], fp32, isOutput=False)
    # weight columns padded to 48: head0 dims at 0:16, head1 dims at 32:48
    wq = nc.declare_dram_parameter("wq", [E, 48], fp32, isOutput=False)
    wk = nc.declare_dram_parameter("wk", [E, 48], fp32, isOutput=False)
    wv = nc.declare_dram_parameter("wv", [E, 48], fp32, isOutput=False)
    maskrow = nc.declare_dram_parameter("maskrow", [1, NK], f32r, isOutput=False)
    onesrow = nc.declare_dram_parameter("onesrow", [1, S], f32r, isOutput=False)
    zrow = nc.declare_dram_parameter("zrow", [1, S], f32r, isOutput=False)
    negm_d = nc.declare_dram_parameter("negm", [2, S], f32r, isOutput=False)
    ident = nc.declare_dram_parameter("ident", [E, E], fp32, isOutput=False)
    out_d = nc.declare_dram_parameter("out", [2 * HD, S], fp32, isOutput=True)
    ldram = nc.dram_tensor("ldram", [2, S], fp32)

    with TileContext(nc) as tc:
        with (
            tc.tile_pool(name="consts", bufs=1) as cpool,
            tc.tile_pool(name="work", bufs=1) as wpool,
            tc.tile_pool(name="ptp", bufs=3) as ptpool,
            tc.tile_pool(name="stp", bufs=2, space="PSUM") as stpool,
            tc.tile_pool(name="ap", bufs=2, space="PSUM") as apool,
            tc.tile_pool(name="ctxp", bufs=2, space="PSUM") as ctxpool,
        ):
            # ---------------- input loads first (sync-queue order) ----------
            xT_sb = cpool.tile([E, S], fp32, name="xT_sb")
            wq_sb = cpool.tile([E, 48], fp32, name="wq_sb")
            wk_sb = cpool.tile([E, 48], fp32, name="wk_sb")
            wv_sb = cpool.tile([E, 48], fp32, name="wv_sb")
            xkT_sb = cpool.tile([E, NK], fp32, name="xkT_sb")
            ident_sb = cpool.tile([E, E], fp32, name="ident_sb")
            nc.sync.dma_start(out=xT_sb[:, 0:512], in_=xT[:, 0:512])
            nc.sync.dma_start(out=wq_sb[:, :], in_=wq[:, :])
            nc.sync.dma_start(out=xkT_sb[:, 0:512], in_=xkT[:, 0:512])
            nc.sync.dma_start(out=wk_sb[:, :], in_=wk[:, :])
            nc.sync.dma_start(out=xT_sb[:, 512:1024], in_=xT[:, 512:1024])
            nc.sync.dma_start(out=ident_sb[:, :], in_=ident[:, :])

            def rest_inputs():
                for o in range(512, NK, 512):
                    n = min(512, NK - o)
                    nc.sync.dma_start(out=xkT_sb[:, o : o + n], in_=xkT[:, o : o + n])
                nc.sync.dma_start(out=xT_sb[:, 1024:1536], in_=xT[:, 1024:1536])
                nc.sync.dma_start(out=xT_sb[:, 1536:2048], in_=xT[:, 1536:2048])
                nc.sync.dma_start(out=wv_sb[:, :], in_=wv[:, :])

            # ---------------- persistent work tensors ----------------
            qt = [wpool.tile([128, S], f32r, name=f"qt{h}") for h in range(2)]
            kt = [wpool.tile([128, NK], f32r, name=f"kt{h}") for h in range(2)]
            qhb48 = wpool.tile([48, S], bf16, name="qhb48")
            khb48 = wpool.tile([48, NK], bf16, name="khb48")
            negid = wpool.tile([48, 48], bf16, name="negid")
            vv = [wpool.tile([128, NKB, HD + 1], f32r, name=f"vv{h}") for h in range(2)]
            ctxl = wpool.tile([49, S], fp32, name="ctxl")
            lsq = wpool.tile([128, 2 * HD], fp32, name="lsq")
            lisq = wpool.tile([128, 2 * HD], fp32, name="lisq")
            lbc = wpool.tile([48, S], fp32, name="lbc")
            out_sb = wpool.tile([64, S], fp32, name="out_sb")

            # special rows (zero fills are issued later, after the first
            # projection loads, to keep the DMA queues free for the critical
            # input loads)
            def specials(h):
                nc.sync.dma_start(out=qt[h][16:17, :], in_=onesrow[:, :])
                nc.sync.dma_start(out=qt[h][17:18, :], in_=negm_d[h : h + 1, :])
                nc.sync.dma_start(out=kt[h][16:17, :], in_=maskrow[:, :])
                nc.sync.dma_start(out=kt[h][17:18, :], in_=onesrow[:, 0:NK])
                nc.sync.dma_start(
                    out=vv[h][:, :, HD : HD + 1],
                    in_=onesrow[0:1, 0:NKB].to_broadcast([128, NKB]),
                )

            nc.scalar.mul(negid[:, :], ident_sb[0:48, 0:48], -1.0)

            def zero_fills(h):
                # zero the gap rows (both sides: 0 * 0 avoids NaN from stale
                # SBUF).  Only true gaps are zeroed, so there are no WAW
                # dependencies against the split/dup writes.
                for lo, hi in ((18, 32), (48, 64)):
                    nc.sync.dma_start(
                        out=qt[h][lo:hi, :],
                        in_=zrow[0:1, 0:S].to_broadcast([hi - lo, S]),
                    )
                    nc.sync.dma_start(
                        out=kt[h][lo:hi, :],
                        in_=zrow[0:1, 0:NK].to_broadcast([hi - lo, NK]),
                    )

            # ---------------- projections + splits ----------------
            # Q: 1/sqrt(hd) folded into wq host-side.  Per 512-col chunk:
            def q_chunk(ci, pool=None, tag=None):
                pool = pool or apool
                cs = slice(512 * ci, 512 * (ci + 1))
                qt_ps = pool.tile([48, 512], fp32, name="qt_ps", tag=tag or "a")
                nc.tensor.matmul(
                    qt_ps[:, :], lhsT=wq_sb[:, :], rhs=xT_sb[:, cs], start=True, stop=False
                )
                nc.scalar.copy(qhb48[:, cs], qt_ps[:, :])  # bf16 round
                nc.tensor.matmul(                                     # PSUM -= Qh -> Ql
                    qt_ps[:, :], lhsT=negid[:, :], rhs=qhb48[:, cs], start=False, stop=True
                )
                for h in range(2):
                    nc.vector.tensor_copy(
                        out=qt[h][0:16, cs], in_=qhb48[32 * h : 32 * h + 16, cs]
                    )
                    nc.vector.tensor_copy(
                        out=qt[h][64:80, cs], in_=qt_ps[32 * h : 32 * h + 16, :]
                    )

            def k_proj(o, n, pool=None, tag=None):
                pool = pool or apool
                cs = slice(o, o + n)
                kt_ps = pool.tile([48, 512], fp32, name="kt_ps", tag=tag or "a")
                nc.tensor.matmul(
                    kt_ps[:, 0:n], lhsT=wk_sb[:, :], rhs=xkT_sb[:, cs], start=True, stop=False
                )
                nc.scalar.copy(khb48[:, cs], kt_ps[:, 0:n])  # bf16 round
                nc.tensor.matmul(                                     # PSUM -= Kh -> Kl
                    kt_ps[:, 0:n], lhsT=negid[:, :], rhs=khb48[:, cs], start=False, stop=True
                )
                for h in range(2):
                    nc.vector.tensor_copy(
                        out=kt[h][0:16, cs], in_=khb48[32 * h : 32 * h + 16, cs]
                    )
                    nc.vector.tensor_copy(
                        out=kt[h][32:48, cs], in_=kt_ps[32 * h : 32 * h + 16, 0:n]
                    )

            def q_dup(h, half):
                cs = slice(1024 * half, 1024 * (half + 1))
                nc.sync.dma_start(out=qt[h][32:48, cs], in_=qt[h][0:16, cs])

            def k_dup(h, lo, hi):
                nc.sync.dma_start(out=kt[h][64:80, lo:hi], in_=kt[h][0:16, lo:hi])

            def v_iter(kb):
                v_ps = apool.tile([128, 48], fp32, name="v_ps", tag="a")
                nc.tensor.matmul(
                    v_ps[:, :],
                    lhsT=xkT_sb[:, 128 * kb : 128 * (kb + 1)],
                    rhs=wv_sb[:, :],
                    start=True,
                    stop=True,
                )
                nc.vector.tensor_copy(out=vv[0][:, kb, 0:HD], in_=v_ps[:, 0:16])
                nc.vector.tensor_copy(out=vv[1][:, kb, 0:HD], in_=v_ps[:, 32:48])

            # ---------------- pass B + ctx (software-pipelined) --------
            def st_exp(h, qh, kb):
                st = stpool.tile([128, 1024], fp32, name="st", tag="st")
                lhs = kt[h][0:80, 128 * kb : 128 * (kb + 1)]
                for c in range(2):
                    nc.tensor.matmul(
                        st[:, 512 * c : 512 * (c + 1)],
                        lhsT=lhs,
                        rhs=qt[h][0:80, 1024 * qh + 512 * c : 1024 * qh + 512 * (c + 1)],
                        start=True,
                        stop=True,
                    )
                pt = ptpool.tile([128, 1024], f32r, name="pt", tag="pt")
                nc.scalar.activation(pt[:, :], st[:, :], AF.Exp)
                return pt

            def ctx_mm(h, kb, ctxc, pt):
                for c in range(2):
                    nc.tensor.matmul(
                        ctxc[c][0:17, :],
                        lhsT=vv[h][:, kb, :],
                        rhs=pt[:, 512 * c : 512 * (c + 1)],
                        start=(kb == 0),
                        stop=(kb == NKB - 1),
                    )

            def evac(h, qh, ctxc):
                for c in range(2):
                    nc.vector.tensor_copy(
                        out=ctxl[
                            32 * h : 32 * h + 17,
                            1024 * qh + 512 * c : 1024 * qh + 512 * (c + 1),
                        ],
                        in_=ctxc[c][0:17, :],
                    )

            def b_half(h, qh):
                return [
                    ctxpool.tile([17, 512], fp32, name=f"ctx{c}", tag="ctx")
                    for c in range(2)
                ]

            def finals(h, qh, ctxc=None):
                # l half-row -> [128,8] square, parallel reciprocal, back out
                q0 = 1024 * qh
                co = 16 * h + 8 * qh
                if False:
                    pass
                else:
                    nc.sync.dma_start(
                        out=lsq[:, co : co + 8],
                        in_=ctxl[32 * h + 16 : 32 * h + 17, q0 : q0 + 1024].rearrange(
                            "a (b f) -> a b f", b=128
                        ),
                    )
                nc.vector.reciprocal(lisq[:, co : co + 8], lsq[:, co : co + 8])
                nc.sync.dma_start(
                    out=ldram[h : h + 1, q0 : q0 + 1024].rearrange(
                        "a (b f) -> a b f", b=128
                    ),
                    in_=lisq[:, co : co + 8],
                )
                nc.sync.dma_start(
                    out=lbc[32 * h : 32 * h + 16, q0 : q0 + 1024],
                    in_=ldram[h : h + 1, q0 : q0 + 1024].to_broadcast([HD, 1024]),
                )
                nc.vector.tensor_tensor(
                    out=out_sb[32 * h : 32 * h + 16, q0 : q0 + 1024],
                    in0=ctxl[32 * h : 32 * h + 16, q0 : q0 + 1024],
                    in1=lbc[32 * h : 32 * h + 16, q0 : q0 + 1024],
                    op=ALU.mult,
                )
                nc.sync.dma_start(
                    out=out_d[16 * h : 16 * h + 16, q0 : q0 + 1024],
                    in_=out_sb[32 * h : 32 * h + 16, q0 : q0 + 1024],
                )

            # ---------------- schedule ----------------
            specials(0)
            zero_fills(0)
            rest_inputs()
            specials(1)
            q_chunk(0, stpool, "st")
            k_proj(0, 512)
            q_chunk(1, stpool, "st")
            for h in range(2):
                k_dup(h, 0, 512)
                q_dup(h, 0)
            zero_fills(1)
            v_iter(0)
            v_iter(1)
            KCH = tuple((o, min(512, NK - o)) for o in range(512, NK, 512))

            def b_stream(h, qh, inter, vmax=0):
                ctxc = b_half(h, qh)
                prev = None
                for kb in range(NKB):
                    pt = st_exp(h, qh, kb)
                    if prev is not None:
                        ctx_mm(h, kb - 1, ctxc, prev)
                    prev = pt
                    if kb + 2 < vmax:
                        v_iter(kb + 2)
                    npop = min(len(inter), max(1, -(-len(inter) // (NKB - kb))))
                    for _ in range(npop):
                        inter.pop(0)()
                ctx_mm(h, NKB - 1, ctxc, prev)
                while inter:
                    inter.pop(0)()
                return ctxc

            inter = []
            for o, n in KCH:
                inter.append(lambda o=o, n=n: k_proj(o, n))
                inter.append(lambda o=o, n=n: [k_dup(h, o, o + n) for h in range(2)])
            inter += [lambda: q_chunk(2), lambda: q_chunk(3)]
            inter += [lambda: [q_dup(h, 1) for h in range(2)]]
            ctxc = b_stream(0, 0, inter, vmax=NKB)
            evac(0, 0, ctxc)

            ctxc = b_stream(0, 1, [])
            evac(0, 1, ctxc)
            finals(0, 0)

            ctxc = b_stream(1, 0, [])
            evac(1, 0, ctxc)
            finals(0, 1)

            ctxc = b_stream(1, 1, [])
            finals(1, 0)
            evac(1, 1, ctxc)
            finals(1, 1)

    nc.finalize()
    return nc


def _prep_core_inputs(x, msk_add_full, w_query, w_key, w_value):
    """Build the 8 per-core input maps from full inputs.  Returns (maps, NKB)."""
    B = x.shape[0]
    onesrow = np.ones((1, S), dtype=np.float32)
    zrow = np.zeros((1, S), dtype=np.float32)
    identm = np.eye(E, dtype=np.float32)

    keeps = [np.flatnonzero(msk_add_full[b] == 0.0) for b in range(B)]
    max_nk = max(len(k) for k in keeps)
    assert max_nk >= NA, "row-max subsample needs >= NA valid keys"
    NKB = -(-max_nk // 128)  # ceil to 128
    NK = 128 * NKB

    per_batch = []
    for b in range(B):
        keep = keeps[b]
        nk = len(keep)
        xk = np.zeros((NK, E), dtype=np.float32)
        xk[:nk] = x[b][keep]
        maskrow = np.full((1, NK), NEG, dtype=np.float32)
        maskrow[0, :nk] = 0.0
        xTb = np.ascontiguousarray(x[b].T)
        xkTb = np.ascontiguousarray(xk.T)
        per_batch.append((xTb, xkTb, maskrow))

    # Cauchy-Schwarz softmax shift: m_hat = 0.25*|q_row|*max_k|k| - 30
    qn = [None, None]
    kmax = np.zeros((B, 8), dtype=np.float64)
    for b in range(B):
        xb = x[b].astype(np.float64)
        qf = (xb @ w_query.astype(np.float64)) * 0.25   # [S, E]
        kf = x[b][keeps[b]].astype(np.float64) @ w_key.astype(np.float64)
        qn[b] = np.stack(
            [np.linalg.norm(qf[:, h::8], axis=1) for h in range(8)], axis=0
        )  # [8, S]
        kmax[b] = np.stack(
            [np.linalg.norm(kf[:, h::8], axis=1).max() for h in range(8)]
        )

    in_maps = []
    for c in range(8):
        b = c // 4
        h0 = 2 * (c % 4)
        xTb, xkTb, maskrow = per_batch[b]
        negm = np.stack(
            [
                -(qn[b][h0] * kmax[b][h0] - 30.0),
                -(qn[b][h0 + 1] * kmax[b][h0 + 1] - 30.0),
            ]
        ).astype(np.float32)  # [2, S]

        def _pad48(w, scale=1.0):
            wc = np.zeros((E, 48), dtype=np.float32)
            wc[:, 0:16] = w[:, h0::8] * scale
            wc[:, 32:48] = w[:, h0 + 1 :: 8] * scale
            return wc

        in_maps.append(
            {
                "xT": xTb,
                "xkT": xkTb,
                "wq": _pad48(w_query, 0.25),  # 1/sqrt(hd) folded in (exact)
                "wk": _pad48(w_key),
                "wv": _pad48(w_value),
                "maskrow": maskrow,
                "onesrow": onesrow,
                "zrow": zrow,
                "negm": negm,
                "ident": identm,
            }
        )
    return in_maps, NKB


def kernel(
    input_embeddings,
    token_attention_masks_source,
    token_attention_masks_target,
    masked,
    w_query,
    w_key,
    w_value,
):
    x = np.asarray(input_embeddings, dtype=np.float32)
    msk = np.asarray(token_attention_masks_source)
    wq_f = np.asarray(w_query, dtype=np.float32)
    wk_f = np.asarray(w_key, dtype=np.float32)
    wv_f = np.asarray(w_value, dtype=np.float32)
    assert int(np.asarray(masked)) == 0, "only the encoder (masked=0) path is supported"
    B = x.shape[0]
    assert x.shape == (2, S, E)

    msk_add = np.where(msk == 0, np.float32(NEG), np.float32(0.0))
    in_maps, NKB = _prep_core_inputs(x, msk_add, wq_f, wk_f, wv_f)

    if NKB not in _PROGS:
        _PROGS[NKB] = _build_program(NKB)
    nc = _PROGS[NKB]
    global _PROG
    _PROG = nc

    from concourse.bass_utils import run_bass_kernel_spmd

    res = run_bass_kernel_spmd(nc, in_maps, list(range(8)))

    out = np.empty((B, S, E), dtype=np.float32)
    for c in range(8):
        b = c // 4
        h0 = 2 * (c % 4)
        o = res.results[c]["out"]  # [32, 2048]
        out[b][:, h0::8] = o[0:16, :].T
        out[b][:, h0 + 1 :: 8] = o[16:32, :].T

    # The device row-max is a lower bound from a 512-key subsample; rows where
    # the true max exceeds it by >~88 overflow exp to inf (-> inf or NaN or,
    # when only the denominator overflows, an exact-zero vector).  Those rows
    # are deterministic and rare (<1%); recompute them exactly on host.
    for b in range(B):
        for h in range(8):
            hv = out[b][:, h::8]  # [S, 16]
            bad = ~np.isfinite(hv).all(axis=1) | (hv == 0.0).all(axis=1)
            if not bad.any():
                continue
            rows = np.flatnonzero(bad)
            xb = x[b].astype(np.float64)
            qh = (xb[rows] @ wq_f[:, h::8].astype(np.float64)) * 0.25
            kh = xb @ wk_f[:, h::8].astype(np.float64)
            vh = xb @ wv_f[:, h::8].astype(np.float64)
            sc = qh @ kh.T + msk_add[b][None, :].astype(np.float64)
            sc -= sc.max(axis=1, keepdims=True)
            p = np.exp(sc)
            p /= p.sum(axis=1, keepdims=True)
            out[b][rows, h::8] = (p @ vh).astype(np.float32)
    return out


_PROG = None


# revision 38
# speedup vs baseline: 1.0297x; 1.0297x over previous
"""Multi-head attention (B=2, H=8, S=2048, hd=16) on 8 Trainium2 NeuronCores.

Sharding: 16 (batch, head) groups -> 2 heads per core (cores 0-3: batch 0,
cores 4-7: batch 1).  Each core gets transposed embeddings, a key-compacted
copy (keys with source-mask 0 dropped; padded to NK with -1000 mask columns),
and the 32 projection-weight columns for its two heads.

Score matmuls run in float32r (1 cycle/row on the PE vs 4 for fp32) with
fp32-level accuracy recovered via split-precision row packing: K and Q are
each split into bf16-high + fp32-residual parts (Kh+Kl, Qh+Ql) and the four
cross products are packed into one 128-row contraction
  rows  0:16  Kh x Qh        rows 32:48  Kl x Qh(dup)
  rows 64:80  Kh(dup) x Ql   (Kl x Ql dropped: ~4e-4 score error)
  row 16: mask x ones        row 17: ones x (-rowmax)      (gaps zeroed)
Extra contraction rows are free (matmul cost is N output columns only), and
bf16-grid values pass through the PE's f32r truncation unchanged, so the sum
reconstructs the exact fp32 product.

The softmax shift per query row is a host-computed Cauchy-Schwarz upper
bound m = 0.25*|q_row|*max_k|k| - 30: it can never overflow exp (bound >=
true max, 30 margin under fp32's e^88), and underflow of the whole row
requires max-cos < 0.4 over ~1k random keys (P ~ e^-58).  No on-device
row-max pass at all; inf/NaN/zero rows would be repaired on host (safety
net, expected zero).

ctx = P^T @ [V | 1] accumulates in PSUM with f32r operands; the ones column
gives the softmax denominator l; 1/l is computed on a [128,16] reshape (not
the serial [1,2048] row) and applied via DRAM-broadcast + DVE multiply.
Output per core is [32, 2048] (dim-major); the host scatters back into the
interleaved head layout.
"""

import numpy as np

S = 2048
E = 128
HD = 16
NQB = S // 128       # 16 query blocks
NEG = -1000.0
NA = 512             # keys sampled for the row-max pass

_PROGS = {}


def _build_program(NKB):
    import concourse.mybir as mybir
    from concourse import bacc
    from concourse.tile import TileContext

    NK = 128 * NKB

    fp32 = mybir.dt.float32
    f32r = mybir.dt.float32r
    bf16 = mybir.dt.bfloat16
    AF = mybir.ActivationFunctionType
    ALU = mybir.AluOpType
    AX = mybir.AxisListType

    nc = bacc.Bacc()

    xT = nc.declare_dram_parameter("xT", [E, S], fp32, isOutput=False)
    xkT = nc.declare_dram_parameter("xkT", [E, NK], fp32, isOutput=False)
    # weight columns padded to 48: head0 dims at 0:16, head1 dims at 32:48
    wq = nc.declare_dram_parameter("wq", [E, 48], fp32, isOutput=False)
    wk = nc.declare_dram_parameter("wk", [E, 48], fp32, isOutput=False)
    wv = nc.declare_dram_parameter("wv", [E, 48], fp32, isOutput=False)
    maskrow = nc.declare_dram_parameter("maskrow", [1, NK], f32r, isOutput=False)
    onesrow = nc.declare_dram_parameter("onesrow", [1, S], f32r, isOutput=False)
    zrow = nc.declare_dram_parameter("zrow", [1, S], f32r, isOutput=False)
    negm_d = nc.declare_dram_parameter("negm", [2, S], f32r, isOutput=False)
    ident = nc.declare_dram_parameter("ident", [E, E], fp32, isOutput=False)
    out_d = nc.declare_dram_parameter("out", [2 * HD, S], fp32, isOutput=True)
    ldram = nc.dram_tensor("ldram", [2, S], fp32)

    with TileContext(nc) as tc:
        with (
            tc.tile_pool(name="consts", bufs=1) as cpool,
            tc.tile_pool(name="work", bufs=1) as wpool,
            tc.tile_pool(name="ptp", bufs=3) as ptpool,
            tc.tile_pool(name="stp", bufs=2, space="PSUM") as stpool,
            tc.tile_pool(name="ap", bufs=2, space="PSUM") as apool,
            tc.tile_pool(name="ctxp", bufs=2, space="PSUM") as ctxpool,
        ):
            # ---------------- input loads first (sync-queue order) ----------
            xT_sb = cpool.tile([E, S], fp32, name="xT_sb")
            wq_sb = cpool.tile([E, 48], fp32, name="wq_sb")
            wk_sb = cpool.tile([E, 48], fp32, name="wk_sb")
            wv_sb = cpool.tile([E, 48], fp32, name="wv_sb")
            xkT_sb = cpool.tile([E, NK], fp32, name="xkT_sb")
            ident_sb = cpool.tile([E, E], fp32, name="ident_sb")
            nc.sync.dma_start(out=xT_sb[:, 0:512], in_=xT[:, 0:512])
            nc.sync.dma_start(out=wq_sb[:, :], in_=wq[:, :])
            nc.sync.dma_start(out=xkT_sb[:, 0:512], in_=xkT[:, 0:512])
            nc.sync.dma_start(out=wk_sb[:, :], in_=wk[:, :])
            nc.sync.dma_start(out=xT_sb[:, 512:1024], in_=xT[:, 512:1024])
            nc.sync.dma_start(out=ident_sb[:, :], in_=ident[:, :])

            def rest_inputs():
                for o in range(512, NK, 512):
                    n = min(512, NK - o)
                    nc.sync.dma_start(out=xkT_sb[:, o : o + n], in_=xkT[:, o : o + n])
                nc.sync.dma_start(out=xT_sb[:, 1024:1536], in_=xT[:, 1024:1536])
                nc.sync.dma_start(out=xT_sb[:, 1536:2048], in_=xT[:, 1536:2048])
                nc.sync.dma_start(out=wv_sb[:, :], in_=wv[:, :])

            # ---------------- persistent work tensors ----------------
            qt = [wpool.tile([128, S], f32r, name=f"qt{h}") for h in range(2)]
            kt = [wpool.tile([128, NK], f32r, name=f"kt{h}") for h in range(2)]
            qhb48 = wpool.tile([48, S], bf16, name="qhb48")
            khb48 = wpool.tile([48, NK], bf16, name="khb48")
            negid = wpool.tile([48, 48], bf16, name="negid")
            vv = [wpool.tile([128, NKB, HD + 1], f32r, name=f"vv{h}") for h in range(2)]
            ctxl = wpool.tile([49, S], fp32, name="ctxl")
            lsq = wpool.tile([128, 2 * HD], fp32, name="lsq")
            lisq = wpool.tile([128, 2 * HD], fp32, name="lisq")
            lbc = wpool.tile([48, S], fp32, name="lbc")
            out_sb = wpool.tile([64, S], fp32, name="out_sb")

            # special rows (zero fills are issued later, after the first
            # projection loads, to keep the DMA queues free for the critical
            # input loads)
            def specials(h):
                nc.sync.dma_start(out=qt[h][16:17, :], in_=onesrow[:, :])
                nc.sync.dma_start(out=qt[h][17:18, :], in_=negm_d[h : h + 1, :])
                nc.sync.dma_start(out=kt[h][16:17, :], in_=maskrow[:, :])
                nc.sync.dma_start(out=kt[h][17:18, :], in_=onesrow[:, 0:NK])
                nc.sync.dma_start(
                    out=vv[h][:, :, HD : HD + 1],
                    in_=onesrow[0:1, 0:NKB].to_broadcast([128, NKB]),
                )

            nc.scalar.mul(negid[:, :], ident_sb[0:48, 0:48], -1.0)

            def zero_fills(h):
                # zero the gap rows (both sides: 0 * 0 avoids NaN from stale
                # SBUF).  Only true gaps are zeroed, so there are no WAW
                # dependencies against the split/dup writes.
                for lo, hi in ((18, 32), (48, 64)):
                    nc.sync.dma_start(
                        out=qt[h][lo:hi, :],
                        in_=zrow[0:1, 0:S].to_broadcast([hi - lo, S]),
                    )
                    nc.sync.dma_start(
                        out=kt[h][lo:hi, :],
                        in_=zrow[0:1, 0:NK].to_broadcast([hi - lo, NK]),
                    )

            # ---------------- projections + splits ----------------
            # Q: 1/sqrt(hd) folded into wq host-side.  Per 512-col chunk:
            def q_chunk(ci, pool=None, tag=None):
                pool = pool or apool
                cs = slice(512 * ci, 512 * (ci + 1))
                qt_ps = pool.tile([48, 512], fp32, name="qt_ps", tag=tag or "a")
                nc.tensor.matmul(
                    qt_ps[:, :], lhsT=wq_sb[:, :], rhs=xT_sb[:, cs], start=True, stop=False
                )
                nc.scalar.copy(qhb48[:, cs], qt_ps[:, :])  # bf16 round
                nc.tensor.matmul(                                     # PSUM -= Qh -> Ql
                    qt_ps[:, :], lhsT=negid[:, :], rhs=qhb48[:, cs], start=False, stop=True
                )
                for h in range(2):
                    nc.vector.tensor_copy(
                        out=qt[h][0:16, cs], in_=qhb48[32 * h : 32 * h + 16, cs]
                    )
                    nc.vector.tensor_copy(
                        out=qt[h][64:80, cs], in_=qt_ps[32 * h : 32 * h + 16, :]
                    )

            def k_proj(o, n, pool=None, tag=None):
                pool = pool or apool
                cs = slice(o, o + n)
                kt_ps = pool.tile([48, 512], fp32, name="kt_ps", tag=tag or "a")
                nc.tensor.matmul(
                    kt_ps[:, 0:n], lhsT=wk_sb[:, :], rhs=xkT_sb[:, cs], start=True, stop=False
                )
                nc.scalar.copy(khb48[:, cs], kt_ps[:, 0:n])  # bf16 round
                nc.tensor.matmul(                                     # PSUM -= Kh -> Kl
                    kt_ps[:, 0:n], lhsT=negid[:, :], rhs=khb48[:, cs], start=False, stop=True
                )
                for h in range(2):
                    nc.vector.tensor_copy(
                        out=kt[h][0:16, cs], in_=khb48[32 * h : 32 * h + 16, cs]
                    )
                    nc.vector.tensor_copy(
                        out=kt[h][32:48, cs], in_=kt_ps[32 * h : 32 * h + 16, 0:n]
                    )

            def q_dup(h, half):
                cs = slice(1024 * half, 1024 * (half + 1))
                nc.sync.dma_start(out=qt[h][32:48, cs], in_=qt[h][0:16, cs])

            def k_dup(h, lo, hi):
                nc.sync.dma_start(out=kt[h][64:80, lo:hi], in_=kt[h][0:16, lo:hi])

            def v_iter(kb):
                v_ps = apool.tile([128, 48], fp32, name="v_ps", tag="a")
                nc.tensor.matmul(
                    v_ps[:, :],
                    lhsT=xkT_sb[:, 128 * kb : 128 * (kb + 1)],
                    rhs=wv_sb[:, :],
                    start=True,
                    stop=True,
                )
                nc.vector.tensor_copy(out=vv[0][:, kb, 0:HD], in_=v_ps[:, 0:16])
                nc.vector.tensor_copy(out=vv[1][:, kb, 0:HD], in_=v_ps[:, 32:48])

            # ---------------- pass B + ctx (software-pipelined) --------
            def st_exp(h, qh, kb):
                st = stpool.tile([128, 1024], fp32, name="st", tag="st")
                lhs = kt[h][0:80, 128 * kb : 128 * (kb + 1)]
                for c in range(2):
                    nc.tensor.matmul(
                        st[:, 512 * c : 512 * (c + 1)],
                        lhsT=lhs,
                        rhs=qt[h][0:80, 1024 * qh + 512 * c : 1024 * qh + 512 * (c + 1)],
                        start=True,
                        stop=True,
                    )
                pt = ptpool.tile([128, 1024], f32r, name="pt", tag="pt")
                nc.scalar.activation(pt[:, :], st[:, :], AF.Exp)
                return pt

            def ctx_mm(h, kb, ctxc, pt):
                for c in range(2):
                    nc.tensor.matmul(
                        ctxc[c][0:17, :],
                        lhsT=vv[h][:, kb, :],
                        rhs=pt[:, 512 * c : 512 * (c + 1)],
                        start=(kb == 0),
                        stop=(kb == NKB - 1),
                    )

            def evac(h, qh, ctxc):
                for c in range(2):
                    nc.vector.tensor_copy(
                        out=ctxl[
                            32 * h : 32 * h + 17,
                            1024 * qh + 512 * c : 1024 * qh + 512 * (c + 1),
                        ],
                        in_=ctxc[c][0:17, :],
                    )

            def b_half(h, qh):
                return [
                    ctxpool.tile([17, 512], fp32, name=f"ctx{c}", tag="ctx")
                    for c in range(2)
                ]

            def finals(h, qh, ctxc=None):
                # l half-row -> [128,8] square, parallel reciprocal, back out
                q0 = 1024 * qh
                co = 16 * h + 8 * qh
                if False:
                    pass
                else:
                    nc.sync.dma_start(
                        out=lsq[:, co : co + 8],
                        in_=ctxl[32 * h + 16 : 32 * h + 17, q0 : q0 + 1024].rearrange(
                            "a (b f) -> a b f", b=128
                        ),
                    )
                nc.vector.reciprocal(lisq[:, co : co + 8], lsq[:, co : co + 8])
                nc.sync.dma_start(
                    out=ldram[h : h + 1, q0 : q0 + 1024].rearrange(
                        "a (b f) -> a b f", b=128
                    ),
                    in_=lisq[:, co : co + 8],
                )
                nc.sync.dma_start(
                    out=lbc[32 * h : 32 * h + 16, q0 : q0 + 1024],
                    in_=ldram[h : h + 1, q0 : q0 + 1024].to_broadcast([HD, 1024]),
                )
                nc.vector.tensor_tensor(
                    out=out_sb[32 * h : 32 * h + 16, q0 : q0 + 1024],
                    in0=ctxl[32 * h : 32 * h + 16, q0 : q0 + 1024],
                    in1=lbc[32 * h : 32 * h + 16, q0 : q0 + 1024],
                    op=ALU.mult,
                )
                nc.sync.dma_start(
                    out=out_d[16 * h : 16 * h + 16, q0 : q0 + 1024],
                    in_=out_sb[32 * h : 32 * h + 16, q0 : q0 + 1024],
                )

            # ---------------- schedule ----------------
            specials(0)
            zero_fills(0)
            rest_inputs()
            specials(1)
            q_chunk(0, stpool, "st")
            k_proj(0, 512)
            q_chunk(1, stpool, "st")
            for h in range(2):
                k_dup(h, 0, 512)
                q_dup(h, 0)
            zero_fills(1)
            KCH = tuple((o, min(512, NK - o)) for o in range(512, NK, 512))
            for o, n in KCH:
                k_proj(o, n)
                for h in range(2):
                    k_dup(h, o, o + n)
            q_chunk(2)
            q_chunk(3)
            for h in range(2):
                q_dup(h, 1)
            for kb in range(NKB):
                v_iter(kb)

            def b_stream(h, qh, inter, vmax=0):
                ctxc = b_half(h, qh)
                prev = None
                for kb in range(NKB):
                    pt = st_exp(h, qh, kb)
                    if prev is not None:
                        ctx_mm(h, kb - 1, ctxc, prev)
                    prev = pt
                    if kb + 2 < vmax:
                        v_iter(kb + 2)
                    npop = min(len(inter), max(1, -(-len(inter) // (NKB - kb))))
                    for _ in range(npop):
                        inter.pop(0)()
                ctx_mm(h, NKB - 1, ctxc, prev)
                while inter:
                    inter.pop(0)()
                return ctxc

            ctxc = b_stream(0, 0, [])
            evac(0, 0, ctxc)

            ctxc = b_stream(0, 1, [])
            evac(0, 1, ctxc)
            finals(0, 0)

            ctxc = b_stream(1, 0, [])
            evac(1, 0, ctxc)
            finals(0, 1)

            ctxc = b_stream(1, 1, [])
            finals(1, 0)
            evac(1, 1, ctxc)
            finals(1, 1)

    nc.finalize()
    return nc


def _prep_core_inputs(x, msk_add_full, w_query, w_key, w_value):
    """Build the 8 per-core input maps from full inputs.  Returns (maps, NKB)."""
    B = x.shape[0]
    onesrow = np.ones((1, S), dtype=np.float32)
    zrow = np.zeros((1, S), dtype=np.float32)
    identm = np.eye(E, dtype=np.float32)

    keeps = [np.flatnonzero(msk_add_full[b] == 0.0) for b in range(B)]
    max_nk = max(len(k) for k in keeps)
    assert max_nk >= NA, "row-max subsample needs >= NA valid keys"
    NKB = -(-max_nk // 128)  # ceil to 128
    NK = 128 * NKB

    per_batch = []
    for b in range(B):
        keep = keeps[b]
        nk = len(keep)
        xk = np.zeros((NK, E), dtype=np.float32)
        xk[:nk] = x[b][keep]
        maskrow = np.full((1, NK), NEG, dtype=np.float32)
        maskrow[0, :nk] = 0.0
        xTb = np.ascontiguousarray(x[b].T)
        xkTb = np.ascontiguousarray(xk.T)
        per_batch.append((xTb, xkTb, maskrow))

    # Cauchy-Schwarz softmax shift: m_hat = 0.25*|q_row|*max_k|k| - 30
    qn = [None, None]
    kmax = np.zeros((B, 8), dtype=np.float64)
    for b in range(B):
        xb = x[b].astype(np.float64)
        qf = (xb @ w_query.astype(np.float64)) * 0.25   # [S, E]
        kf = x[b][keeps[b]].astype(np.float64) @ w_key.astype(np.float64)
        qn[b] = np.stack(
            [np.linalg.norm(qf[:, h::8], axis=1) for h in range(8)], axis=0
        )  # [8, S]
        kmax[b] = np.stack(
            [np.linalg.norm(kf[:, h::8], axis=1).max() for h in range(8)]
        )

    in_maps = []
    for c in range(8):
        b = c // 4
        h0 = 2 * (c % 4)
        xTb, xkTb, maskrow = per_batch[b]
        negm = np.stack(
            [
                -(qn[b][h0] * kmax[b][h0] - 30.0),
                -(qn[b][h0 + 1] * kmax[b][h0 + 1] - 30.0),
            ]
        ).astype(np.float32)  # [2, S]

        def _pad48(w, scale=1.0):
            wc = np.zeros((E, 48), dtype=np.float32)
            wc[:, 0:16] = w[:, h0::8] * scale
            wc[:, 32:48] = w[:, h0 + 1 :: 8] * scale
            return wc

        in_maps.append(
            {
                "xT": xTb,
                "xkT": xkTb,
                "wq": _pad48(w_query, 0.25),  # 1/sqrt(hd) folded in (exact)
                "wk": _pad48(w_key),
                "wv": _pad48(w_value),
                "maskrow": maskrow,
                "onesrow": onesrow,
                "zrow": zrow,
                "negm": negm,
                "ident": identm,
            }
        )
    return in_maps, NKB


def kernel(
    input_embeddings,
    token_attention_masks_source,
    token_attention_masks_target,
    masked,
    w_query,
    w_key,
    w_value,
):
    x = np.asarray(input_embeddings, dtype=np.float32)
    msk = np.asarray(token_attention_masks_source)
    wq_f = np.asarray(w_query, dtype=np.float32)
    wk_f = np.asarray(w_key, dtype=np.float32)
    wv_f = np.asarray(w_value, dtype=np.float32)
    assert int(np.asarray(masked)) == 0, "only the encoder (masked=0) path is supported"
    B = x.shape[0]
    assert x.shape == (2, S, E)

    msk_add = np.where(msk == 0, np.float32(NEG), np.float32(0.0))
    in_maps, NKB = _prep_core_inputs(x, msk_add, wq_f, wk_f, wv_f)

    if NKB not in _PROGS:
        _PROGS[NKB] = _build_program(NKB)
    nc = _PROGS[NKB]
    global _PROG
    _PROG = nc

    from concourse.bass_utils import run_bass_kernel_spmd

    res = run_bass_kernel_spmd(nc, in_maps, list(range(8)))

    out = np.empty((B, S, E), dtype=np.float32)
    for c in range(8):
        b = c // 4
        h0 = 2 * (c % 4)
        o = res.results[c]["out"]  # [32, 2048]
        out[b][:, h0::8] = o[0:16, :].T
        out[b][:, h0 + 1 :: 8] = o[16:32, :].T

    # The device row-max is a lower bound from a 512-key subsample; rows where
    # the true max exceeds it by >~88 overflow exp to inf (-> inf or NaN or,
    # when only the denominator overflows, an exact-zero vector).  Those rows
    # are deterministic and rare (<1%); recompute them exactly on host.
    for b in range(B):
        for h in range(8):
            hv = out[b][:, h::8]  # [S, 16]
            bad = ~np.isfinite(hv).all(axis=1) | (hv == 0.0).all(axis=1)
            if not bad.any():
                continue
            rows = np.flatnonzero(bad)
            xb = x[b].astype(np.float64)
            qh = (xb[rows] @ wq_f[:, h::8].astype(np.float64)) * 0.25
            kh = xb @ wk_f[:, h::8].astype(np.float64)
            vh = xb @ wv_f[:, h::8].astype(np.float64)
            sc = qh @ kh.T + msk_add[b][None, :].astype(np.float64)
            sc -= sc.max(axis=1, keepdims=True)
            p = np.exp(sc)
            p /= p.sum(axis=1, keepdims=True)
            out[b][rows, h::8] = (p @ vh).astype(np.float32)
    return out


_PROG = None
